# revision 34
# baseline (speedup 1.0000x reference)
"""Single-head attention (B=8, S=4096, E=512, H=64) on 8 trn2 NeuronCores.

Sharding: data-parallel over batch — one batch element per core.

Per-core algorithm (batch b), v4:
  - Host pre-transposes x[b] -> xT [E, S] in bf16 and converts the int32
    mask to fp8e4m3 {0,1}, pre-tiled into per-group [128, 4, 2, 128]
    slabs (16 MB; one contiguous DMA per score group, prefetched MLOOK
    groups ahead so phase B never waits on mask DMA — v1 idled the PE
    ~80us at phase-B start waiting for its big mask tiles).
  - QKV: Q^T,K^T [H, S] head-major and V' [S, H+1] S-major (ones column
    appended), all bf16, via PE matmuls over E-chunks.
  - Scores computed TRANSPOSED: S^T[sk, sq] = K^T.T @ Q^T so softmax runs
    along partitions and attn @ V needs no transpose of attn.
  - Mask applied additively PRE-exp using the PE's free lhsT transpose
    with the mask chunk STATIONARY and a constant -240*I as the MOVING
    operand: S^T += mask_chunk.T @ (-240 * I), fp8 on both sides.
    POWER NOTE: this shape is chosen deliberately.  The trn2 activity
    governor (ham windows, util limit 0.5 every ~3.4us) clamps the PE to
    ~57% rate when sustained streaming power is too high.  Streaming the
    dense mask as the moving operand (v3) clamps the whole phase B;
    loading the mask as weights and streaming a 99%-zeros diagonal keeps
    window power low.  bf16 (not f32r) scores/attn@V for the same
    reason: f32r streams at ~1.4x the power of bf16 and v3 (f32r) ran
    clamped start to finish at 3411 ns/group vs 1731 free.
    All elementwise two-tensor ops (DVE tensor_tensor /
    scalar_tensor_tensor, Pool tensor_tensor) are avoided: they
    pair-accumulate on lanes 84-95/116-127 under concurrent load.
  - exp on ACT with no max-subtraction (|scaled scores| < ~10, safe),
    bf16 out; exp(scale*(qk - 240*m)) = 2.5e-5 * w on masked lanes —
    negligible against unmasked softmax mass (rel ~3e-5).
  - Softmax denominator comes free from the ones column of V':
    outT = V'.T @ attn^T accumulates [H+1, sq] where row H is the row sum.
  - Fixup per q block: 4 batched PE transposes into ONE PSUM bank tile,
    reciprocal + scale on DVE (per-partition-scalar ops only), one
    gathered DMA out.

Phase B runs as one flat pipeline over all 128 (qb, g) groups with attn@V
trailing the scores/exp chain by TRAIL groups.
"""
import sys

sys.path.insert(0, "/opt/trn_rl_repo")

import ml_dtypes
import numpy as np

import concourse.bacc as bacc
import concourse.tile as tile
from concourse import mybir
from concourse.bass_utils import run_bass_kernel_spmd

F32 = mybir.dt.float32
BF16 = mybir.dt.bfloat16
FP8 = mybir.dt.float8e4

B, S, E, H = 8, 4096, 512, 64
SCALE = float(E) ** -0.5
NEG = -240.0  # max-magnitude finite fp8e4m3 (IEEE): exactly representable

BF16NP = ml_dtypes.bfloat16
FP8NP = ml_dtypes.float8_e4m3

HTRAIL = 4  # attn@V trails scores/exp by this many [128,512] half-groups
MLOOK = 12  # mask DMA prefetch depth, in groups
N_IDLE_DMA = 10  # chained 512KB dummy DMAs idling the PE before phase B


def build_program(s=S):
    nc = bacc.Bacc("TRN2", target_bir_lowering=False, debug=False, num_devices=B)
    NE = E // 128          # 4 E-chunks
    NB = s // 512          # q/s blocks of 512
    NQ = s // 128          # 128-row chunks
    NG = NQ // 2           # [128,1024]-score groups per q block
    GQ = NB * NG           # total groups

    xT = nc.dram_tensor("xT", [E, s], BF16, kind="ExternalInput")
    # mask, pre-tiled per group: row block (g*NB + qb)*128 holds that
    # group's [128, 4(j), 2(h2), 128] slab contiguously
    maskt = nc.dram_tensor("maskt", [NG * NB * 128, 4, 2, 128], FP8,
                           kind="ExternalInput")
    wq = nc.dram_tensor("wq", [E, H], BF16, kind="ExternalInput")
    wk = nc.dram_tensor("wk", [E, H], BF16, kind="ExternalInput")
    wv = nc.dram_tensor("wv", [E, H], BF16, kind="ExternalInput")
    bqt = nc.dram_tensor("bqt", [H, 1], F32, kind="ExternalInput")
    bkt = nc.dram_tensor("bkt", [H, 1], F32, kind="ExternalInput")
    bv1 = nc.dram_tensor("bv1", [1, H + 1], BF16, kind="ExternalInput")
    out = nc.dram_tensor("out", [s, H], F32, kind="ExternalOutput")

    with tile.TileContext(nc) as tc:
        with (
            tc.tile_pool(name="const", bufs=1) as cst,
            tc.tile_pool(name="xp", bufs=3) as xp,
            tc.tile_pool(name="qkv", bufs=1) as qkv,
            tc.tile_pool(name="maskp", bufs=MLOOK) as maskp,
            tc.tile_pool(name="etp", bufs=6) as etp,
            tc.tile_pool(name="osb", bufs=2) as osb,
        ):
            # ---- constants ----
            # negI is built at the END of phase A, gated behind a chain of
            # dummy DMAs (see below): the PE sits mostly idle ~15us before
            # phase B, which parks the activity governor's power average
            # well below its clamp threshold.  Without this the same
            # kernel coin-flips between free-running (286us) and spending
            # 150us+ clamped at half rate (347us) depending on where the
            # hysteresis state lands at phase-B entry.
            negI = cst.tile([128, 128], FP8)
            idf = cst.tile([128, 128], F32)
            nc.gpsimd.memset(idf, 0.0)
            nc.gpsimd.affine_select(
                out=idf, in_=idf, compare_op=mybir.AluOpType.not_equal,
                fill=1.0, base=0, pattern=[[-1, 128]], channel_multiplier=1,
            )
            ones128 = cst.tile([1, 128], BF16)
            nc.vector.memset(ones128, 1.0)

            wq_r = cst.tile([128, NE, H], BF16)
            wk_r = cst.tile([128, NE, H], BF16)
            wv_r = cst.tile([128, NE, H], BF16)
            for w_dram, w_r in ((wq, wq_r), (wk, wk_r), (wv, wv_r)):
                nc.sync.dma_start(
                    out=w_r, in_=w_dram.rearrange("(c p) h -> p c h", p=128)
                )
            bv1_sb = cst.tile([1, H + 1], BF16)
            nc.sync.dma_start(out=bv1_sb, in_=bv1[:])
            bqt_sb = cst.tile([H, 1], F32)
            bkt_sb = cst.tile([H, 1], F32)
            nc.sync.dma_start(out=bqt_sb, in_=bqt[:])
            nc.sync.dma_start(out=bkt_sb, in_=bkt[:])

            # ---- mask DMA: per-group [128, 4, 2, 128] fp8 slabs ----
            mtiles = {}

            def mask_dma(G, eng):
                qb, g = divmod(G, NG)
                r0 = (g * NB + qb) * 128
                mt = maskp.tile([128, 4, 2, 128], FP8, tag="mt", name=f"mt_{G}")
                eng.dma_start(out=mt, in_=maskt[r0:r0 + 128, :, :, :])
                mtiles[G] = mt

            # ---- phase A: QT, KT head-major; V' S-major (all bf16) ----
            # (fp8 q/k was tried: rel err 2.45e-2 > 2e-2 gate, and N=512
            # matmul streaming is width-bound, not dtype-bound, so fp8
            # bought no speed either.)
            QTb = [qkv.tile([H, 512], BF16, name=f"qt_{i}") for i in range(NB)]
            KTb = [qkv.tile([H, 512], BF16, name=f"kt_{i}") for i in range(NB)]
            VPk = [qkv.tile([128, H + 1], BF16, name=f"vp_{i}") for i in range(NQ)]
            def xtr_dma(sb):
                s0 = sb * 512
                xtr = xp.tile([128, NE, 512], BF16, tag="xtr", name=f"xtr_{sb}")
                half = NE // 2
                for eh in range(2):
                    e0 = eh * half
                    eng = nc.sync if eh == 0 else nc.gpsimd
                    eng.dma_start(
                        out=xtr[:, e0:e0 + half, :],
                        in_=xT[e0 * 128:(e0 + half) * 128, s0:s0 + 512]
                        .rearrange("(c p) s -> p c s", p=128),
                    )
                return xtr

            # x tiles for sb 0/1 first on scalar/gpsimd queues; mask
            # prefetch ramps on the sync queue only during phase A.
            xtrs = {0: xtr_dma(0), 1: xtr_dma(1), 2: xtr_dma(2)}
            for G in range(2):
                mask_dma(G, nc.sync)

            with tc.tile_pool(name="psA", bufs=2, space="PSUM") as psA:
                for sb in range(NB):
                    for G in range(2 + sb * 2, 2 + sb * 2 + 2):
                        if G < MLOOK:
                            mask_dma(G, nc.sync)
                    if sb + 3 < NB:
                        xtrs[sb + 3] = xtr_dma(sb + 3)
                    xtr = xtrs.pop(sb)
                    q_ps = psA.tile([H, 512], F32, tag="qk", name=f"q_ps_{sb}",
                                    bufs=4)
                    k_ps = psA.tile([H, 512], F32, tag="qk", name=f"k_ps_{sb}",
                                    bufs=4)
                    for e in range(NE):
                        nc.tensor.matmul(q_ps, wq_r[:, e, :], xtr[:, e, :],
                                         start=(e == 0), stop=(e == NE - 1))
                        nc.tensor.matmul(k_ps, wk_r[:, e, :], xtr[:, e, :],
                                         start=(e == 0), stop=(e == NE - 1))
                    nc.scalar.activation(QTb[sb], q_ps,
                                         mybir.ActivationFunctionType.Identity,
                                         bias=bqt_sb)
                    nc.scalar.activation(KTb[sb], k_ps,
                                         mybir.ActivationFunctionType.Identity,
                                         bias=bkt_sb)
                    for j0 in range(0, 4, 2):
                        vps = [
                            psA.tile([128, H + 1], F32, tag="v",
                                     name=f"v_ps_{sb}_{j0 + jj}")
                            for jj in range(2)
                        ]
                        for jj in range(2):
                            nc.tensor.matmul(vps[jj], ones128, bv1_sb,
                                             start=True, stop=False)
                        for e in range(NE):
                            for jj in range(2):
                                c0 = (j0 + jj) * 128
                                nc.tensor.matmul(
                                    vps[jj][:, 0:H], xtr[:, e, c0:c0 + 128],
                                    wv_r[:, e, :], start=False, stop=(e == NE - 1),
                                )
                        for jj in range(2):
                            nc.vector.tensor_copy(VPk[sb * 4 + j0 + jj], vps[jj])

            # ---- deliberate idle: bank governor credit before phase B ----
            # gate: DVE copy reads the last V' tile (ready only at phase A
            # end), then N_IDLE_DMA chained 512KB DMAs (WAW on one tile,
            # ~1.5-2us each) run with the PE idle, then negI is built so
            # the first mask matmul (and with it steady phase B) waits.
            dummy = cst.tile([128, 2048], F32)
            nc.vector.tensor_copy(dummy[:, 0:H + 1], VPk[NQ - 1])
            for _ in range(N_IDLE_DMA):
                nc.scalar.dma_start(
                    out=dummy.bitcast(BF16)[:, 0:2048],
                    in_=xT[0:128, 0:2048],
                )
            nc.vector.tensor_copy(negI[:, 0:4], dummy[:, 0:4])
            nc.gpsimd.memset(negI, 0.0)
            nc.gpsimd.affine_select(
                out=negI, in_=negI, compare_op=mybir.AluOpType.not_equal,
                fill=NEG, base=0, pattern=[[-1, 128]], channel_multiplier=1,
            )

            # ---- phase B: flat pipeline over all half-groups ----
            # Each [128, 512] score half-tile is ONE PSUM bank; exp and
            # attn@V consume per half so banks recycle twice as fast as
            # the [128,1024] two-bank variant (which stalled scores ~315ns
            # per group on bank reuse).
            with (
                tc.tile_pool(name="psS", bufs=6, space="PSUM") as psS,
                tc.tile_pool(name="psO", bufs=1, space="PSUM") as psO,
            ):
                ot_ps = [None] * NB
                HQ = 2 * GQ  # half-groups; HG -> qb = HG//32, k = HG%32

                def scoresH(HG):
                    # NOTE: chunking these into 4x128-wide matmuls was
                    # tried (v9): the denser schedule trips the activity
                    # governor and the whole run clamps to ~half rate.
                    # The 512-wide form runs at the power-sustainable pace.
                    qb, k = HG // (2 * NG), HG % (2 * NG)
                    G, h2 = divmod(HG, 2)
                    sc = psS.tile([128, 512], F32, tag="sc", name=f"sc_{HG}")
                    mt = mtiles[G]
                    nc.tensor.matmul(
                        sc,
                        KTb[k // 4][:, 128 * (k % 4):128 * (k % 4 + 1)],
                        QTb[qb],
                        start=True, stop=False,
                    )
                    for j in range(4):
                        nc.tensor.matmul(
                            sc[:, 128 * j:128 * (j + 1)],
                            mt[:, j, h2, :], negI,
                            start=False, stop=(j == 3),
                        )
                    if h2 == 1:
                        mtiles.pop(G)
                    return sc

                def expH(HG, sc):
                    et = etp.tile([128, 512], BF16, tag="et")
                    nc.scalar.activation(
                        et, sc, mybir.ActivationFunctionType.Exp, scale=SCALE
                    )
                    return et

                def attnvH(HG, et):
                    qb, k = HG // (2 * NG), HG % (2 * NG)
                    if ot_ps[qb] is None:
                        ot_ps[qb] = psO.tile([H + 1, 512], F32, tag="ot",
                                             name=f"ot_{qb}")
                    nc.tensor.matmul(
                        ot_ps[qb], VPk[k], et,
                        start=(k == 0), stop=(k == NQ - 1),
                    )

                def fixup(qb):
                    q0 = qb * 512
                    oT = osb.tile([H + 1, 512], F32, tag="oT")
                    nc.vector.tensor_copy(oT, ot_ps[qb])
                    fx = psS.tile([128, 4, H + 1], F32, tag="fx", bufs=1)
                    for j in range(4):
                        nc.tensor.transpose(
                            fx[:, j, :], oT[:, 128 * j:128 * (j + 1)],
                            idf[0:H + 1, 0:H + 1]
                        )
                    ob = osb.tile([128, 4, H + 1], F32, tag="ob")
                    nc.vector.tensor_copy(ob, fx)
                    rc = osb.tile([128, 4], F32, tag="rc")
                    nc.vector.reciprocal(rc, ob[:, :, H])
                    of = osb.tile([128, 4, H], F32, tag="of")
                    for j in range(4):
                        nc.vector.tensor_scalar_mul(
                            of[:, j, :], ob[:, j, 0:H], rc[:, j:j + 1]
                        )
                    nc.gpsimd.dma_start(
                        out=out[q0:q0 + 512, :].rearrange("(j p) h -> p j h", p=128),
                        in_=of,
                    )

                KH = 2 * NG  # half-groups per qb
                ets = {}
                scn = {0: scoresH(0), 1: scoresH(1)}
                for HG in range(HQ):
                    if HG % 2 == 0 and HG // 2 + MLOOK < GQ:
                        mask_dma(HG // 2 + MLOOK,
                                 nc.sync if HG % 4 == 0 else nc.gpsimd)
                    if HG + 2 < HQ:
                        scn[HG + 2] = scoresH(HG + 2)
                    ets[HG] = expH(HG, scn.pop(HG))
                    if HG - HTRAIL >= 0:
                        attnvH(HG - HTRAIL, ets.pop(HG - HTRAIL))
                        if (HG - HTRAIL) % KH == KH - 1:
                            fixup((HG - HTRAIL) // KH)
                for HG in range(HQ - HTRAIL, HQ):
                    attnvH(HG, ets.pop(HG))
                    if HG % KH == KH - 1:
                        fixup(HG // KH)
    nc.compile()
    return nc


def make_in_maps(x, attention_mask, Wq, bq, Wk, bk, Wv, bv):
    nb = x.shape[0]
    NG, NB = S // 256, S // 512
    bv1 = np.concatenate([bv, np.ones(1, np.float32)]).reshape(1, H + 1)
    common = {
        "wq": np.ascontiguousarray(Wq.astype(BF16NP)),
        "wk": np.ascontiguousarray(Wk.astype(BF16NP)),
        "wv": np.ascontiguousarray(Wv.astype(BF16NP)),
        "bqt": np.ascontiguousarray(bq.reshape(H, 1)),
        "bkt": np.ascontiguousarray(bk.reshape(H, 1)),
        "bv1": bv1.astype(BF16NP),
    }
    in_maps = []
    for b in range(nb):
        # mask -> fp8 {0,1} bytes (1.0 == 0x38 in e4m3), pre-tiled to
        # [(g, qb, p), j, h2, c]: mask[qb*512 + j*128 + p, (2g+h2)*128 + c]
        m8 = ((attention_mask[b] != 0).astype(np.uint8) * np.uint8(0x38))
        mt = m8.reshape(NB, 4, 128, NG, 2, 128).transpose(3, 0, 2, 1, 4, 5)
        mt = np.ascontiguousarray(mt).reshape(NG * NB * 128, 4, 2, 128)
        in_maps.append({
            "xT": np.ascontiguousarray(x[b].T.astype(BF16NP)),
            "maskt": mt.view(FP8NP),
            **common,
        })
    return in_maps


_PROGRAM = None


def kernel(x, attention_mask, Wq, bq, Wk, bk, Wv, bv):
    global _PROGRAM
    x = np.asarray(x, np.float32)
    attention_mask = np.asarray(attention_mask, np.int32)
    if _PROGRAM is None:
        _PROGRAM = build_program()
    in_maps = make_in_maps(
        x, attention_mask,
        np.asarray(Wq, np.float32), np.asarray(bq, np.float32),
        np.asarray(Wk, np.float32), np.asarray(bk, np.float32),
        np.asarray(Wv, np.float32), np.asarray(bv, np.float32),
    )
    res = run_bass_kernel_spmd(_PROGRAM, in_maps, core_ids=list(range(B)))
    return np.stack([res.results[b]["out"] for b in range(B)], axis=0)


# revision 37
# speedup vs baseline: 1.1732x; 1.1732x over previous
"""Single-head attention (B=8, S=4096, E=512, H=64) on 8 trn2 NeuronCores.

Sharding: data-parallel over batch — one batch element per core.

Per-core algorithm (batch b), v4:
  - Host pre-transposes x[b] -> xT [E, S] in bf16 and converts the int32
    mask to fp8e4m3 {0,1}, pre-tiled into per-group [128, 4, 2, 128]
    slabs (16 MB; one contiguous DMA per score group, prefetched MLOOK
    groups ahead so phase B never waits on mask DMA — v1 idled the PE
    ~80us at phase-B start waiting for its big mask tiles).
  - QKV: Q^T,K^T [H, S] head-major and V' [S, H+1] S-major (ones column
    appended), all bf16, via PE matmuls over E-chunks.
  - Scores computed TRANSPOSED: S^T[sk, sq] = K^T.T @ Q^T so softmax runs
    along partitions and attn @ V needs no transpose of attn.
  - Mask applied additively PRE-exp using the PE's free lhsT transpose
    with the mask chunk STATIONARY and a constant -240*I as the MOVING
    operand: S^T += mask_chunk.T @ (-240 * I), fp8 on both sides.
    POWER NOTE: this shape is chosen deliberately.  The trn2 activity
    governor (ham windows, util limit 0.5 every ~3.4us) clamps the PE to
    ~57% rate when sustained streaming power is too high.  Streaming the
    dense mask as the moving operand (v3) clamps the whole phase B;
    loading the mask as weights and streaming a 99%-zeros diagonal keeps
    window power low.  bf16 (not f32r) scores/attn@V for the same
    reason: f32r streams at ~1.4x the power of bf16 and v3 (f32r) ran
    clamped start to finish at 3411 ns/group vs 1731 free.
    All elementwise two-tensor ops (DVE tensor_tensor /
    scalar_tensor_tensor, Pool tensor_tensor) are avoided: they
    pair-accumulate on lanes 84-95/116-127 under concurrent load.
  - exp on ACT with no max-subtraction (|scaled scores| < ~10, safe),
    bf16 out; exp(scale*(qk - 240*m)) = 2.5e-5 * w on masked lanes —
    negligible against unmasked softmax mass (rel ~3e-5).
  - Softmax denominator comes free from the ones column of V':
    outT = V'.T @ attn^T accumulates [H+1, sq] where row H is the row sum.
  - Fixup per q block: 4 batched PE transposes into ONE PSUM bank tile,
    reciprocal + scale on DVE (per-partition-scalar ops only), one
    gathered DMA out.

Phase B runs as one flat pipeline over all 128 (qb, g) groups with attn@V
trailing the scores/exp chain by TRAIL groups.
"""
import sys

sys.path.insert(0, "/opt/trn_rl_repo")

import ml_dtypes
import numpy as np

import concourse.bacc as bacc
import concourse.tile as tile
from concourse import mybir
from concourse.bass_utils import run_bass_kernel_spmd

F32 = mybir.dt.float32
BF16 = mybir.dt.bfloat16
FP8 = mybir.dt.float8e4

B, S, E, H = 8, 4096, 512, 64
SCALE = float(E) ** -0.5
NEG = -240.0  # max-magnitude finite fp8e4m3 (IEEE): exactly representable

BF16NP = ml_dtypes.bfloat16
FP8NP = ml_dtypes.float8_e4m3

HTRAIL = 4  # attn@V trails scores/exp by this many [128,512] half-groups
MLOOK = 12  # mask DMA prefetch depth, in groups



def build_program(s=S):
    nc = bacc.Bacc("TRN2", target_bir_lowering=False, debug=False, num_devices=B)
    NE = E // 128          # 4 E-chunks
    NB = s // 512          # q/s blocks of 512
    NQ = s // 128          # 128-row chunks
    NG = NQ // 2           # [128,1024]-score groups per q block
    GQ = NB * NG           # total groups

    xT = nc.dram_tensor("xT", [E, s], BF16, kind="ExternalInput")
    # mask, pre-tiled per group: row block (g*NB + qb)*128 holds that
    # group's [128, 4(j), 2(h2), 128] slab contiguously
    maskt = nc.dram_tensor("maskt", [NG * NB * 128, 4, 2, 128], FP8,
                           kind="ExternalInput")
    wq = nc.dram_tensor("wq", [E, H], BF16, kind="ExternalInput")
    wk = nc.dram_tensor("wk", [E, H], BF16, kind="ExternalInput")
    wv = nc.dram_tensor("wv", [E, H], BF16, kind="ExternalInput")
    bqt = nc.dram_tensor("bqt", [H, 1], F32, kind="ExternalInput")
    bkt = nc.dram_tensor("bkt", [H, 1], F32, kind="ExternalInput")
    bv1 = nc.dram_tensor("bv1", [1, H + 1], BF16, kind="ExternalInput")
    out = nc.dram_tensor("out", [s, H], F32, kind="ExternalOutput")

    with tile.TileContext(nc) as tc:
        with (
            tc.tile_pool(name="const", bufs=1) as cst,
            tc.tile_pool(name="xp", bufs=3) as xp,
            tc.tile_pool(name="qkv", bufs=1) as qkv,
            tc.tile_pool(name="maskp", bufs=MLOOK) as maskp,
            tc.tile_pool(name="etp", bufs=6) as etp,
            tc.tile_pool(name="osb", bufs=2) as osb,
        ):
            # ---- constants ----
            negI = cst.tile([128, 128], FP8)
            nc.gpsimd.memset(negI, 0.0)
            nc.gpsimd.affine_select(
                out=negI, in_=negI, compare_op=mybir.AluOpType.not_equal,
                fill=NEG, base=0, pattern=[[-1, 128]], channel_multiplier=1,
            )
            idf = cst.tile([128, 128], F32)
            nc.gpsimd.memset(idf, 0.0)
            nc.gpsimd.affine_select(
                out=idf, in_=idf, compare_op=mybir.AluOpType.not_equal,
                fill=1.0, base=0, pattern=[[-1, 128]], channel_multiplier=1,
            )
            ones128 = cst.tile([1, 128], BF16)
            nc.vector.memset(ones128, 1.0)

            wq_r = cst.tile([128, NE, H], BF16)
            wk_r = cst.tile([128, NE, H], BF16)
            wv_r = cst.tile([128, NE, H], BF16)
            for w_dram, w_r in ((wq, wq_r), (wk, wk_r), (wv, wv_r)):
                nc.sync.dma_start(
                    out=w_r, in_=w_dram.rearrange("(c p) h -> p c h", p=128)
                )
            bv1_sb = cst.tile([1, H + 1], BF16)
            nc.sync.dma_start(out=bv1_sb, in_=bv1[:])
            bqt_sb = cst.tile([H, 1], F32)
            bkt_sb = cst.tile([H, 1], F32)
            nc.sync.dma_start(out=bqt_sb, in_=bqt[:])
            nc.sync.dma_start(out=bkt_sb, in_=bkt[:])

            # ---- mask DMA: per-group [128, 4, 2, 128] fp8 slabs ----
            mtiles = {}

            def mask_dma(G, eng):
                qb, g = divmod(G, NG)
                r0 = (g * NB + qb) * 128
                mt = maskp.tile([128, 4, 2, 128], FP8, tag="mt", name=f"mt_{G}")
                eng.dma_start(out=mt, in_=maskt[r0:r0 + 128, :, :, :])
                mtiles[G] = mt

            # ---- phase A: QT, KT head-major; V' S-major (all bf16) ----
            # (fp8 q/k was tried: rel err 2.45e-2 > 2e-2 gate, and N=512
            # matmul streaming is width-bound, not dtype-bound, so fp8
            # bought no speed either.)
            QTb = [qkv.tile([H, 512], BF16, name=f"qt_{i}") for i in range(NB)]
            KTb = [qkv.tile([H, 512], BF16, name=f"kt_{i}") for i in range(NB)]
            VPk = [qkv.tile([128, H + 1], BF16, name=f"vp_{i}") for i in range(NQ)]
            def xtr_dma(sb):
                s0 = sb * 512
                xtr = xp.tile([128, NE, 512], BF16, tag="xtr", name=f"xtr_{sb}")
                half = NE // 2
                for eh in range(2):
                    e0 = eh * half
                    eng = nc.sync if eh == 0 else nc.gpsimd
                    eng.dma_start(
                        out=xtr[:, e0:e0 + half, :],
                        in_=xT[e0 * 128:(e0 + half) * 128, s0:s0 + 512]
                        .rearrange("(c p) s -> p c s", p=128),
                    )
                return xtr

            # x tiles for sb 0/1 first on scalar/gpsimd queues; mask
            # prefetch ramps on the sync queue only during phase A.
            xtrs = {0: xtr_dma(0), 1: xtr_dma(1), 2: xtr_dma(2)}
            for G in range(2):
                mask_dma(G, nc.sync)

            with tc.tile_pool(name="psA", bufs=2, space="PSUM") as psA:
                for sb in range(NB):
                    for G in range(2 + sb * 2, 2 + sb * 2 + 2):
                        if G < MLOOK:
                            mask_dma(G, nc.sync)
                    if sb + 3 < NB:
                        xtrs[sb + 3] = xtr_dma(sb + 3)
                    xtr = xtrs.pop(sb)
                    q_ps = psA.tile([H, 512], F32, tag="qk", name=f"q_ps_{sb}",
                                    bufs=4)
                    k_ps = psA.tile([H, 512], F32, tag="qk", name=f"k_ps_{sb}",
                                    bufs=4)
                    for e in range(NE):
                        nc.tensor.matmul(q_ps, wq_r[:, e, :], xtr[:, e, :],
                                         start=(e == 0), stop=(e == NE - 1))
                        nc.tensor.matmul(k_ps, wk_r[:, e, :], xtr[:, e, :],
                                         start=(e == 0), stop=(e == NE - 1))
                    nc.scalar.activation(QTb[sb], q_ps,
                                         mybir.ActivationFunctionType.Identity,
                                         bias=bqt_sb)
                    nc.scalar.activation(KTb[sb], k_ps,
                                         mybir.ActivationFunctionType.Identity,
                                         bias=bkt_sb)
                    for j0 in range(0, 4, 2):
                        vps = [
                            psA.tile([128, H + 1], F32, tag="v",
                                     name=f"v_ps_{sb}_{j0 + jj}")
                            for jj in range(2)
                        ]
                        for jj in range(2):
                            nc.tensor.matmul(vps[jj], ones128, bv1_sb,
                                             start=True, stop=False)
                        for e in range(NE):
                            for jj in range(2):
                                c0 = (j0 + jj) * 128
                                nc.tensor.matmul(
                                    vps[jj][:, 0:H], xtr[:, e, c0:c0 + 128],
                                    wv_r[:, e, :], start=False, stop=(e == NE - 1),
                                )
                        for jj in range(2):
                            nc.vector.tensor_copy(VPk[sb * 4 + j0 + jj], vps[jj])

            # (A deliberate pre-phase-B idle to appease the activity
            # governor was tried and did NOT prevent clamping — the clamp
            # windows appear driven by package-level state outside this
            # kernel's control, so no time is wasted on idling.)

            # ---- phase B: flat pipeline over all half-groups ----
            # Each [128, 512] score half-tile is ONE PSUM bank; exp and
            # attn@V consume per half so banks recycle twice as fast as
            # the [128,1024] two-bank variant (which stalled scores ~315ns
            # per group on bank reuse).
            with (
                tc.tile_pool(name="psS", bufs=6, space="PSUM") as psS,
                tc.tile_pool(name="psO", bufs=1, space="PSUM") as psO,
            ):
                ot_ps = [None] * NB
                HQ = 2 * GQ  # half-groups; HG -> qb = HG//32, k = HG%32

                def scoresH(HG):
                    # NOTE: chunking these into 4x128-wide matmuls was
                    # tried (v9): the denser schedule trips the activity
                    # governor and the whole run clamps to ~half rate.
                    # The 512-wide form runs at the power-sustainable pace.
                    qb, k = HG // (2 * NG), HG % (2 * NG)
                    G, h2 = divmod(HG, 2)
                    sc = psS.tile([128, 512], F32, tag="sc", name=f"sc_{HG}")
                    mt = mtiles[G]
                    nc.tensor.matmul(
                        sc,
                        KTb[k // 4][:, 128 * (k % 4):128 * (k % 4 + 1)],
                        QTb[qb],
                        start=True, stop=False,
                    )
                    for j in range(4):
                        nc.tensor.matmul(
                            sc[:, 128 * j:128 * (j + 1)],
                            mt[:, j, h2, :], negI,
                            start=False, stop=(j == 3),
                        )
                    if h2 == 1:
                        mtiles.pop(G)
                    return sc

                def expH(HG, sc):
                    et = etp.tile([128, 512], BF16, tag="et")
                    nc.scalar.activation(
                        et, sc, mybir.ActivationFunctionType.Exp, scale=SCALE
                    )
                    return et

                def attnvH(HG, et):
                    qb, k = HG // (2 * NG), HG % (2 * NG)
                    if ot_ps[qb] is None:
                        ot_ps[qb] = psO.tile([H + 1, 512], F32, tag="ot",
                                             name=f"ot_{qb}")
                    nc.tensor.matmul(
                        ot_ps[qb], VPk[k], et,
                        start=(k == 0), stop=(k == NQ - 1),
                    )

                def fixup(qb):
                    q0 = qb * 512
                    oT = osb.tile([H + 1, 512], F32, tag="oT")
                    nc.vector.tensor_copy(oT, ot_ps[qb])
                    fx = psS.tile([128, 4, H + 1], F32, tag="fx", bufs=1)
                    for j in range(4):
                        nc.tensor.transpose(
                            fx[:, j, :], oT[:, 128 * j:128 * (j + 1)],
                            idf[0:H + 1, 0:H + 1]
                        )
                    ob = osb.tile([128, 4, H + 1], F32, tag="ob")
                    nc.vector.tensor_copy(ob, fx)
                    rc = osb.tile([128, 4], F32, tag="rc")
                    nc.vector.reciprocal(rc, ob[:, :, H])
                    of = osb.tile([128, 4, H], F32, tag="of")
                    for j in range(4):
                        nc.vector.tensor_scalar_mul(
                            of[:, j, :], ob[:, j, 0:H], rc[:, j:j + 1]
                        )
                    nc.gpsimd.dma_start(
                        out=out[q0:q0 + 512, :].rearrange("(j p) h -> p j h", p=128),
                        in_=of,
                    )

                KH = 2 * NG  # half-groups per qb
                ets = {}
                scn = {0: scoresH(0), 1: scoresH(1)}
                for HG in range(HQ):
                    if HG % 2 == 0 and HG // 2 + MLOOK < GQ:
                        mask_dma(HG // 2 + MLOOK,
                                 nc.sync if HG % 4 == 0 else nc.gpsimd)
                    if HG + 2 < HQ:
                        scn[HG + 2] = scoresH(HG + 2)
                    ets[HG] = expH(HG, scn.pop(HG))
                    if HG - HTRAIL >= 0:
                        attnvH(HG - HTRAIL, ets.pop(HG - HTRAIL))
                        if (HG - HTRAIL) % KH == KH - 1:
                            fixup((HG - HTRAIL) // KH)
                for HG in range(HQ - HTRAIL, HQ):
                    attnvH(HG, ets.pop(HG))
                    if HG % KH == KH - 1:
                        fixup(HG // KH)
    nc.compile()
    return nc


def make_in_maps(x, attention_mask, Wq, bq, Wk, bk, Wv, bv):
    nb = x.shape[0]
    NG, NB = S // 256, S // 512
    bv1 = np.concatenate([bv, np.ones(1, np.float32)]).reshape(1, H + 1)
    common = {
        "wq": np.ascontiguousarray(Wq.astype(BF16NP)),
        "wk": np.ascontiguousarray(Wk.astype(BF16NP)),
        "wv": np.ascontiguousarray(Wv.astype(BF16NP)),
        "bqt": np.ascontiguousarray(bq.reshape(H, 1)),
        "bkt": np.ascontiguousarray(bk.reshape(H, 1)),
        "bv1": bv1.astype(BF16NP),
    }
    in_maps = []
    for b in range(nb):
        # mask -> fp8 {0,1} bytes (1.0 == 0x38 in e4m3), pre-tiled to
        # [(g, qb, p), j, h2, c]: mask[qb*512 + j*128 + p, (2g+h2)*128 + c]
        m8 = ((attention_mask[b] != 0).astype(np.uint8) * np.uint8(0x38))
        mt = m8.reshape(NB, 4, 128, NG, 2, 128).transpose(3, 0, 2, 1, 4, 5)
        mt = np.ascontiguousarray(mt).reshape(NG * NB * 128, 4, 2, 128)
        in_maps.append({
            "xT": np.ascontiguousarray(x[b].T.astype(BF16NP)),
            "maskt": mt.view(FP8NP),
            **common,
        })
    return in_maps


_PROGRAM = None


def kernel(x, attention_mask, Wq, bq, Wk, bk, Wv, bv):
    global _PROGRAM
    x = np.asarray(x, np.float32)
    attention_mask = np.asarray(attention_mask, np.int32)
    if _PROGRAM is None:
        _PROGRAM = build_program()
    in_maps = make_in_maps(
        x, attention_mask,
        np.asarray(Wq, np.float32), np.asarray(bq, np.float32),
        np.asarray(Wk, np.float32), np.asarray(bk, np.float32),
        np.asarray(Wv, np.float32), np.asarray(bv, np.float32),
    )
    res = run_bass_kernel_spmd(_PROGRAM, in_maps, core_ids=list(range(B)))
    return np.stack([res.results[b]["out"] for b in range(B)], axis=0)


# revision 46
# speedup vs baseline: 1.2211x; 1.0408x over previous
"""Single-head attention (B=8, S=4096, E=512, H=64) on 8 trn2 NeuronCores.

Sharding: data-parallel over batch — one batch element per core.

Per-core algorithm (batch b):
  - Host pre-transposes x[b] -> xT [E, S] in bf16 and converts the int32
    mask to fp8e4m3 {0,1}, pre-tiled into per-group [128, 4, 2, 128]
    slabs (16 MB vs 64 raw; one contiguous DMA per score group,
    prefetched MLOOK groups ahead so phase B never waits on mask DMA —
    the original version idled the PE ~80us at phase-B start waiting for
    its bulk mask tiles).
  - QKV: Q^T,K^T [H, S] head-major and V' [S, H+1] S-major (ones column
    appended), all bf16, via PE matmuls over E-chunks.
  - Scores computed TRANSPOSED: S^T[sk, sq] = K^T.T @ Q^T so softmax runs
    along partitions and attn @ V needs no transpose of attn.
  - Mask applied additively PRE-exp using the PE's free lhsT transpose
    with the mask chunk STATIONARY and a constant -240*I as the MOVING
    operand: S^T += mask_chunk.T @ (-240 * I), fp8 on both sides.
    POWER NOTE: this shape is chosen deliberately.  The trn2 activity
    governor (ham windows, util limit 0.5 every ~3.4us) clamps the PE to
    ~57% rate when sustained streaming power is too high.  Streaming the
    dense mask as the moving operand (v3) clamps the whole phase B;
    loading the mask as weights and streaming a 99%-zeros diagonal keeps
    window power low.  bf16 (not f32r) scores/attn@V for the same
    reason: f32r streams at ~1.4x the power of bf16 and v3 (f32r) ran
    clamped start to finish at 3411 ns/group vs 1731 free.
    All elementwise two-tensor ops (DVE tensor_tensor /
    scalar_tensor_tensor, Pool tensor_tensor) are avoided: they
    pair-accumulate on lanes 84-95/116-127 under concurrent load.
  - exp on ACT with no max-subtraction (|scaled scores| < ~10, safe),
    bf16 out; exp(scale*(qk - 240*m)) = 2.5e-5 * w on masked lanes —
    negligible against unmasked softmax mass (rel ~3e-5).
  - Softmax denominator comes free from the ones column of V':
    outT = V'.T @ attn^T accumulates [H+1, sq] where row H is the row sum.
  - Fixup per q block: 4 batched PE transposes into ONE PSUM bank tile,
    reciprocal + scale on DVE (per-partition-scalar ops only), one
    gathered DMA out.

Phase B runs as one flat pipeline over 256 [128,512] half-groups (each a
single PSUM bank, so banks recycle quickly) with attn@V trailing the
scores/exp chain by HTRAIL half-groups.

Measured: 278-287us (rel err 2.75e-3) when the activity governor leaves
the run alone; ~347us when it clamps (package-state dependent, not
controllable from the kernel — see the power notes above).  Baseline
this session started from: 332us.
"""
import sys

sys.path.insert(0, "/opt/trn_rl_repo")

import ml_dtypes
import numpy as np

import concourse.bacc as bacc
import concourse.tile as tile
from concourse import mybir
from concourse.bass_utils import run_bass_kernel_spmd

F32 = mybir.dt.float32
BF16 = mybir.dt.bfloat16
FP8 = mybir.dt.float8e4

B, S, E, H = 8, 4096, 512, 64
SCALE = float(E) ** -0.5
NEG = -240.0  # max-magnitude finite fp8e4m3 (IEEE): exactly representable

BF16NP = ml_dtypes.bfloat16
FP8NP = ml_dtypes.float8_e4m3

HTRAIL = 2  # attn@V trails scores/exp by this many [128,512] half-groups
MLOOK = 12  # mask DMA prefetch depth, in groups



def build_program(s=S):
    nc = bacc.Bacc("TRN2", target_bir_lowering=False, debug=False, num_devices=B)
    NE = E // 128          # 4 E-chunks
    NB = s // 512          # q/s blocks of 512
    NQ = s // 128          # 128-row chunks
    NG = NQ // 2           # [128,1024]-score groups per q block
    GQ = NB * NG           # total groups

    xT = nc.dram_tensor("xT", [E, s], BF16, kind="ExternalInput")
    # mask, pre-tiled per group: row block (g*NB + qb)*128 holds that
    # group's [128, 4(j), 2(h2), 128] slab contiguously
    maskt = nc.dram_tensor("maskt", [NG * NB * 128, 4, 2, 128], FP8,
                           kind="ExternalInput")
    wq = nc.dram_tensor("wq", [E, H], BF16, kind="ExternalInput")
    wk = nc.dram_tensor("wk", [E, H], BF16, kind="ExternalInput")
    wv = nc.dram_tensor("wv", [E, H], BF16, kind="ExternalInput")
    bqt = nc.dram_tensor("bqt", [H, 1], F32, kind="ExternalInput")
    bkt = nc.dram_tensor("bkt", [H, 1], F32, kind="ExternalInput")
    bvt = nc.dram_tensor("bvt", [H, 1], F32, kind="ExternalInput")
    out = nc.dram_tensor("out", [s, H], F32, kind="ExternalOutput")

    with tile.TileContext(nc) as tc:
        with (
            tc.tile_pool(name="const", bufs=1) as cst,
            tc.tile_pool(name="xp", bufs=3) as xp,
            tc.tile_pool(name="qkv", bufs=1) as qkv,
            tc.tile_pool(name="maskp", bufs=MLOOK) as maskp,
            tc.tile_pool(name="etp", bufs=6) as etp,
            tc.tile_pool(name="osb", bufs=2) as osb,
        ):
            # ---- constants ----
            negI = cst.tile([128, 128], FP8)
            nc.gpsimd.memset(negI, 0.0)
            nc.gpsimd.affine_select(
                out=negI, in_=negI, compare_op=mybir.AluOpType.not_equal,
                fill=NEG, base=0, pattern=[[-1, 128]], channel_multiplier=1,
            )
            idf = cst.tile([128, 128], F32)
            nc.gpsimd.memset(idf, 0.0)
            nc.gpsimd.affine_select(
                out=idf, in_=idf, compare_op=mybir.AluOpType.not_equal,
                fill=1.0, base=0, pattern=[[-1, 128]], channel_multiplier=1,
            )

            wq_r = cst.tile([128, NE, H], BF16)
            wk_r = cst.tile([128, NE, H], BF16)
            wv_r = cst.tile([128, NE, H], BF16)
            for w_dram, w_r in ((wq, wq_r), (wk, wk_r), (wv, wv_r)):
                nc.sync.dma_start(
                    out=w_r, in_=w_dram.rearrange("(c p) h -> p c h", p=128)
                )
            bqt_sb = cst.tile([H, 1], F32)
            bkt_sb = cst.tile([H, 1], F32)
            bvt_sb = cst.tile([H, 1], F32)
            nc.sync.dma_start(out=bqt_sb, in_=bqt[:])
            nc.sync.dma_start(out=bkt_sb, in_=bkt[:])
            nc.sync.dma_start(out=bvt_sb, in_=bvt[:])

            # ---- mask DMA: per-group [128, 4, 2, 128] fp8 slabs ----
            mtiles = {}

            def mask_dma(G, eng):
                qb, g = divmod(G, NG)
                r0 = (g * NB + qb) * 128
                mt = maskp.tile([128, 4, 2, 128], FP8, tag="mt", name=f"mt_{G}")
                eng.dma_start(out=mt, in_=maskt[r0:r0 + 128, :, :, :])
                mtiles[G] = mt

            # ---- phase A: QT, KT head-major; V' S-major (all bf16) ----
            # (fp8 q/k was tried: rel err 2.45e-2 > 2e-2 gate, and N=512
            # matmul streaming is width-bound, not dtype-bound, so fp8
            # bought no speed either.)
            QTb = [qkv.tile([H, 512], BF16, name=f"qt_{i}") for i in range(NB)]
            KTb = [qkv.tile([H, 512], BF16, name=f"kt_{i}") for i in range(NB)]
            VPk = [qkv.tile([128, H + 1], BF16, name=f"vp_{i}") for i in range(NQ)]
            def xtr_dma(sb):
                s0 = sb * 512
                xtr = xp.tile([128, NE, 512], BF16, tag="xtr", name=f"xtr_{sb}")
                half = NE // 2
                for eh in range(2):
                    e0 = eh * half
                    eng = nc.sync if eh == 0 else nc.gpsimd
                    eng.dma_start(
                        out=xtr[:, e0:e0 + half, :],
                        in_=xT[e0 * 128:(e0 + half) * 128, s0:s0 + 512]
                        .rearrange("(c p) s -> p c s", p=128),
                    )
                return xtr

            # x tiles for sb 0/1 first on scalar/gpsimd queues; mask
            # prefetch ramps on the sync queue only during phase A.
            xtrs = {0: xtr_dma(0), 1: xtr_dma(1), 2: xtr_dma(2)}
            for G in range(2):
                mask_dma(G, nc.sync)

            with tc.tile_pool(name="psA", bufs=2, space="PSUM") as psA:
                for sb in range(NB):
                    for G in range(2 + sb * 2, 2 + sb * 2 + 2):
                        if G < MLOOK:
                            mask_dma(G, nc.sync)
                    if sb + 3 < NB:
                        xtrs[sb + 3] = xtr_dma(sb + 3)
                    xtr = xtrs.pop(sb)
                    # q/k/v head-major, 128-wide moving chunks (N=128
                    # chains stream ~0.44 ns/col vs 0.61 at N=512; the
                    # stationary w chunk is constant within a chain so its
                    # LDW hides).  Only the first chunk of each PSUM bank
                    # uses start=True: the 2KB zero-region covers the
                    # whole bank row and later chunks ride pending-zero.
                    q_ps = psA.tile([H, 512], F32, tag="qk", name=f"q_ps_{sb}",
                                    bufs=6)
                    k_ps = psA.tile([H, 512], F32, tag="qk", name=f"k_ps_{sb}",
                                    bufs=6)
                    v_ps = psA.tile([H, 512], F32, tag="qk", name=f"v_ps_{sb}",
                                    bufs=6)
                    for e in range(NE):
                        for ps, w_r in ((q_ps, wq_r), (k_ps, wk_r),
                                        (v_ps, wv_r)):
                            for c in range(4):
                                nc.tensor.matmul(
                                    ps[:, 128 * c:128 * c + 128],
                                    w_r[:, e, :],
                                    xtr[:, e, 128 * c:128 * c + 128],
                                    start=(e == 0 and c == 0),
                                    stop=(e == NE - 1),
                                )
                    nc.scalar.activation(QTb[sb], q_ps,
                                         mybir.ActivationFunctionType.Identity,
                                         bias=bqt_sb)
                    nc.scalar.activation(KTb[sb], k_ps,
                                         mybir.ActivationFunctionType.Identity,
                                         bias=bkt_sb)
                    # V' needs S-major layout: bias via ACT, then 4 PE
                    # transposes into one PSUM bank (sibling-safe pattern
                    # proven by the output fixup), ones column by memset.
                    vt = osb.tile([H, 512], F32, tag="vt")
                    nc.scalar.activation(vt, v_ps,
                                         mybir.ActivationFunctionType.Identity,
                                         bias=bvt_sb)
                    vt_ps = psA.tile([128, 4, H], F32, tag="vtp",
                                     name=f"vt_ps_{sb}", bufs=2)
                    for j in range(4):
                        nc.tensor.transpose(
                            vt_ps[:, j, :], vt[:, 128 * j:128 * (j + 1)],
                            idf[0:H, 0:H]
                        )
                    for j in range(4):
                        vp = VPk[sb * 4 + j]
                        nc.vector.tensor_copy(vp[:, 0:H], vt_ps[:, j, :])
                        nc.vector.memset(vp[:, H:H + 1], 1.0)

            # (A deliberate pre-phase-B idle to appease the activity
            # governor was tried and did NOT prevent clamping — the clamp
            # windows appear driven by package-level state outside this
            # kernel's control, so no time is wasted on idling.)

            # ---- phase B: flat pipeline over all half-groups ----
            # Each [128, 512] score half-tile is ONE PSUM bank; exp and
            # attn@V consume per half so banks recycle twice as fast as
            # the [128,1024] two-bank variant (which stalled scores ~315ns
            # per group on bank reuse).
            with (
                tc.tile_pool(name="psS", bufs=6, space="PSUM") as psS,
                tc.tile_pool(name="psO", bufs=1, space="PSUM") as psO,
            ):
                ot_ps = [None] * NB
                HQ = 2 * GQ  # half-groups; HG -> qb = HG//32, k = HG%32

                def scoresH(HG):
                    # NOTE: chunking these into 4x128-wide matmuls was
                    # tried (v9): the denser schedule trips the activity
                    # governor and the whole run clamps to ~half rate.
                    # The 512-wide form runs at the power-sustainable pace.
                    qb, k = HG // (2 * NG), HG % (2 * NG)
                    G, h2 = divmod(HG, 2)
                    sc = psS.tile([128, 512], F32, tag="sc", name=f"sc_{HG}")
                    mt = mtiles[G]
                    nc.tensor.matmul(
                        sc,
                        KTb[k // 4][:, 128 * (k % 4):128 * (k % 4 + 1)],
                        QTb[qb],
                        start=True, stop=False,
                    )
                    for j in range(4):
                        nc.tensor.matmul(
                            sc[:, 128 * j:128 * (j + 1)],
                            mt[:, j, h2, :], negI,
                            start=False, stop=(j == 3),
                        )
                    if h2 == 1:
                        mtiles.pop(G)
                    return sc

                def expH(HG, sc):
                    et = etp.tile([128, 512], BF16, tag="et")
                    nc.scalar.activation(
                        et, sc, mybir.ActivationFunctionType.Exp, scale=SCALE
                    )
                    return et

                def attnvH(HG, et):
                    qb, k = HG // (2 * NG), HG % (2 * NG)
                    if ot_ps[qb] is None:
                        ot_ps[qb] = psO.tile([H + 1, 512], F32, tag="ot",
                                             name=f"ot_{qb}")
                    nc.tensor.matmul(
                        ot_ps[qb], VPk[k], et,
                        start=(k == 0), stop=(k == NQ - 1),
                    )

                def fixup(qb):
                    q0 = qb * 512
                    oT = osb.tile([H + 1, 512], F32, tag="oT")
                    nc.vector.tensor_copy(oT, ot_ps[qb])
                    fx = psS.tile([128, 4, H + 1], F32, tag="fx", bufs=1)
                    for j in range(4):
                        nc.tensor.transpose(
                            fx[:, j, :], oT[:, 128 * j:128 * (j + 1)],
                            idf[0:H + 1, 0:H + 1]
                        )
                    ob = osb.tile([128, 4, H + 1], F32, tag="ob")
                    nc.vector.tensor_copy(ob, fx)
                    rc = osb.tile([128, 4], F32, tag="rc")
                    nc.vector.reciprocal(rc, ob[:, :, H])
                    of = osb.tile([128, 4, H], F32, tag="of")
                    for j in range(4):
                        nc.vector.tensor_scalar_mul(
                            of[:, j, :], ob[:, j, 0:H], rc[:, j:j + 1]
                        )
                    nc.gpsimd.dma_start(
                        out=out[q0:q0 + 512, :].rearrange("(j p) h -> p j h", p=128),
                        in_=of,
                    )

                KH = 2 * NG  # half-groups per qb
                ets = {}
                scn = {0: scoresH(0), 1: scoresH(1)}
                for HG in range(HQ):
                    if HG % 2 == 0 and HG // 2 + MLOOK < GQ:
                        mask_dma(HG // 2 + MLOOK,
                                 nc.sync if HG % 4 == 0 else nc.gpsimd)
                    if HG + 2 < HQ:
                        scn[HG + 2] = scoresH(HG + 2)
                    ets[HG] = expH(HG, scn.pop(HG))
                    if HG - HTRAIL >= 0:
                        attnvH(HG - HTRAIL, ets.pop(HG - HTRAIL))
                        if (HG - HTRAIL) % KH == KH - 1:
                            fixup((HG - HTRAIL) // KH)
                for HG in range(HQ - HTRAIL, HQ):
                    attnvH(HG, ets.pop(HG))
                    if HG % KH == KH - 1:
                        fixup(HG // KH)
    nc.compile()
    return nc


def make_in_maps(x, attention_mask, Wq, bq, Wk, bk, Wv, bv):
    nb = x.shape[0]
    NG, NB = S // 256, S // 512

    common = {
        "wq": np.ascontiguousarray(Wq.astype(BF16NP)),
        "wk": np.ascontiguousarray(Wk.astype(BF16NP)),
        "wv": np.ascontiguousarray(Wv.astype(BF16NP)),
        "bqt": np.ascontiguousarray(bq.reshape(H, 1)),
        "bkt": np.ascontiguousarray(bk.reshape(H, 1)),
        "bvt": np.ascontiguousarray(bv.reshape(H, 1)),
    }
    in_maps = []
    for b in range(nb):
        # mask -> fp8 {0,1} bytes (1.0 == 0x38 in e4m3), pre-tiled to
        # [(g, qb, p), j, h2, c]: mask[qb*512 + j*128 + p, (2g+h2)*128 + c]
        m8 = ((attention_mask[b] != 0).astype(np.uint8) * np.uint8(0x38))
        mt = m8.reshape(NB, 4, 128, NG, 2, 128).transpose(3, 0, 2, 1, 4, 5)
        mt = np.ascontiguousarray(mt).reshape(NG * NB * 128, 4, 2, 128)
        in_maps.append({
            "xT": np.ascontiguousarray(x[b].T.astype(BF16NP)),
            "maskt": mt.view(FP8NP),
            **common,
        })
    return in_maps


_PROGRAM = None


def kernel(x, attention_mask, Wq, bq, Wk, bk, Wv, bv):
    global _PROGRAM
    x = np.asarray(x, np.float32)
    attention_mask = np.asarray(attention_mask, np.int32)
    if _PROGRAM is None:
        _PROGRAM = build_program()
    in_maps = make_in_maps(
        x, attention_mask,
        np.asarray(Wq, np.float32), np.asarray(bq, np.float32),
        np.asarray(Wk, np.float32), np.asarray(bk, np.float32),
        np.asarray(Wv, np.float32), np.asarray(bv, np.float32),
    )
    res = run_bass_kernel_spmd(_PROGRAM, in_maps, core_ids=list(range(B)))
    return np.stack([res.results[b]["out"] for b in range(B)], axis=0)


# revision 48
# speedup vs baseline: 1.2421x; 1.0172x over previous
"""Single-head attention (B=8, S=4096, E=512, H=64) on 8 trn2 NeuronCores.

Sharding: data-parallel over batch — one batch element per core.

Per-core algorithm (batch b):
  - Host pre-transposes x[b] -> xT [E, S] in bf16 and converts the int32
    mask to fp8e4m3 {0,1}, pre-tiled into per-group [128, 4, 2, 128]
    slabs (16 MB vs 64 raw; one contiguous DMA per score group,
    prefetched MLOOK groups ahead so phase B never waits on mask DMA —
    the original version idled the PE ~80us at phase-B start waiting for
    its bulk mask tiles).
  - QKV: Q^T,K^T [H, S] head-major and V' [S, H+1] S-major (ones column
    appended), all bf16, via PE matmuls over E-chunks.
  - Scores computed TRANSPOSED: S^T[sk, sq] = K^T.T @ Q^T so softmax runs
    along partitions and attn @ V needs no transpose of attn.
  - Mask applied additively PRE-exp using the PE's free lhsT transpose
    with the mask chunk STATIONARY and a constant -240*I as the MOVING
    operand: S^T += mask_chunk.T @ (-240 * I), fp8 on both sides.
    POWER NOTE: this shape is chosen deliberately.  The trn2 activity
    governor (ham windows, util limit 0.5 every ~3.4us) clamps the PE to
    ~57% rate when sustained streaming power is too high.  Streaming the
    dense mask as the moving operand (v3) clamps the whole phase B;
    loading the mask as weights and streaming a 99%-zeros diagonal keeps
    window power low.  bf16 (not f32r) scores/attn@V for the same
    reason: f32r streams at ~1.4x the power of bf16 and v3 (f32r) ran
    clamped start to finish at 3411 ns/group vs 1731 free.
    All elementwise two-tensor ops (DVE tensor_tensor /
    scalar_tensor_tensor, Pool tensor_tensor) are avoided: they
    pair-accumulate on lanes 84-95/116-127 under concurrent load.
  - exp on ACT with no max-subtraction (|scaled scores| < ~10, safe),
    bf16 out; exp(scale*(qk - 240*m)) = 2.5e-5 * w on masked lanes —
    negligible against unmasked softmax mass (rel ~3e-5).
  - Softmax denominator comes free from the ones column of V':
    outT = V'.T @ attn^T accumulates [H+1, sq] where row H is the row sum.
  - Fixup per q block: 4 batched PE transposes into ONE PSUM bank tile,
    reciprocal + scale on DVE (per-partition-scalar ops only), one
    gathered DMA out.

Phase B runs as one flat pipeline over 256 [128,512] half-groups (each a
single PSUM bank, so banks recycle quickly) with attn@V trailing the
scores/exp chain by HTRAIL half-groups.

Measured: 334us (rel err 2.71e-3) with the activity governor clamping
(the same schedule class measured 286us and its predecessor 278us in
earlier, unclamped windows; clamp state is package-level and not
controllable from the kernel — see the power notes above).  The
equivalent kernel without the phase-A restructure measured 347us under
the same clamped conditions.  Baseline this session started from: 332us
(measured in an unclamped window).
"""
import sys

sys.path.insert(0, "/opt/trn_rl_repo")

import ml_dtypes
import numpy as np

import concourse.bacc as bacc
import concourse.tile as tile
from concourse import mybir
from concourse.bass_utils import run_bass_kernel_spmd

F32 = mybir.dt.float32
BF16 = mybir.dt.bfloat16
FP8 = mybir.dt.float8e4

B, S, E, H = 8, 4096, 512, 64
SCALE = float(E) ** -0.5
NEG = -240.0  # max-magnitude finite fp8e4m3 (IEEE): exactly representable

BF16NP = ml_dtypes.bfloat16
FP8NP = ml_dtypes.float8_e4m3

HTRAIL = 4  # attn@V trails scores/exp by this many [128,512] half-groups
            # (2 was tried: steady-state cadence degraded 862 -> 952 ns)
MLOOK = 12  # mask DMA prefetch depth, in groups



def build_program(s=S):
    nc = bacc.Bacc("TRN2", target_bir_lowering=False, debug=False, num_devices=B)
    NE = E // 128          # 4 E-chunks
    NB = s // 512          # q/s blocks of 512
    NQ = s // 128          # 128-row chunks
    NG = NQ // 2           # [128,1024]-score groups per q block
    GQ = NB * NG           # total groups

    xT = nc.dram_tensor("xT", [E, s], BF16, kind="ExternalInput")
    # mask, pre-tiled per group: row block (g*NB + qb)*128 holds that
    # group's [128, 4(j), 2(h2), 128] slab contiguously
    maskt = nc.dram_tensor("maskt", [NG * NB * 128, 4, 2, 128], FP8,
                           kind="ExternalInput")
    wq = nc.dram_tensor("wq", [E, H], BF16, kind="ExternalInput")
    wk = nc.dram_tensor("wk", [E, H], BF16, kind="ExternalInput")
    wv = nc.dram_tensor("wv", [E, H], BF16, kind="ExternalInput")
    bqt = nc.dram_tensor("bqt", [H, 1], F32, kind="ExternalInput")
    bkt = nc.dram_tensor("bkt", [H, 1], F32, kind="ExternalInput")
    bvt = nc.dram_tensor("bvt", [H, 1], F32, kind="ExternalInput")
    out = nc.dram_tensor("out", [s, H], F32, kind="ExternalOutput")

    with tile.TileContext(nc) as tc:
        with (
            tc.tile_pool(name="const", bufs=1) as cst,
            tc.tile_pool(name="xp", bufs=3) as xp,
            tc.tile_pool(name="qkv", bufs=1) as qkv,
            tc.tile_pool(name="maskp", bufs=MLOOK) as maskp,
            tc.tile_pool(name="etp", bufs=6) as etp,
            tc.tile_pool(name="osb", bufs=2) as osb,
        ):
            # ---- constants ----
            negI = cst.tile([128, 128], FP8)
            nc.gpsimd.memset(negI, 0.0)
            nc.gpsimd.affine_select(
                out=negI, in_=negI, compare_op=mybir.AluOpType.not_equal,
                fill=NEG, base=0, pattern=[[-1, 128]], channel_multiplier=1,
            )
            idf = cst.tile([128, 128], F32)
            nc.gpsimd.memset(idf, 0.0)
            nc.gpsimd.affine_select(
                out=idf, in_=idf, compare_op=mybir.AluOpType.not_equal,
                fill=1.0, base=0, pattern=[[-1, 128]], channel_multiplier=1,
            )

            wq_r = cst.tile([128, NE, H], BF16)
            wk_r = cst.tile([128, NE, H], BF16)
            wv_r = cst.tile([128, NE, H], BF16)
            for w_dram, w_r in ((wq, wq_r), (wk, wk_r), (wv, wv_r)):
                nc.sync.dma_start(
                    out=w_r, in_=w_dram.rearrange("(c p) h -> p c h", p=128)
                )
            bqt_sb = cst.tile([H, 1], F32)
            bkt_sb = cst.tile([H, 1], F32)
            bvt_sb = cst.tile([H, 1], F32)
            nc.sync.dma_start(out=bqt_sb, in_=bqt[:])
            nc.sync.dma_start(out=bkt_sb, in_=bkt[:])
            nc.sync.dma_start(out=bvt_sb, in_=bvt[:])

            # ---- mask DMA: per-group [128, 4, 2, 128] fp8 slabs ----
            mtiles = {}

            def mask_dma(G, eng):
                qb, g = divmod(G, NG)
                r0 = (g * NB + qb) * 128
                mt = maskp.tile([128, 4, 2, 128], FP8, tag="mt", name=f"mt_{G}")
                eng.dma_start(out=mt, in_=maskt[r0:r0 + 128, :, :, :])
                mtiles[G] = mt

            # ---- phase A: QT, KT head-major; V' S-major (all bf16) ----
            # (fp8 q/k was tried: rel err 2.45e-2 > 2e-2 gate, and N=512
            # matmul streaming is width-bound, not dtype-bound, so fp8
            # bought no speed either.)
            QTb = [qkv.tile([H, 512], BF16, name=f"qt_{i}") for i in range(NB)]
            KTb = [qkv.tile([H, 512], BF16, name=f"kt_{i}") for i in range(NB)]
            VPk = [qkv.tile([128, H + 1], BF16, name=f"vp_{i}") for i in range(NQ)]
            def xtr_dma(sb):
                s0 = sb * 512
                xtr = xp.tile([128, NE, 512], BF16, tag="xtr", name=f"xtr_{sb}")
                half = NE // 2
                for eh in range(2):
                    e0 = eh * half
                    eng = nc.sync if eh == 0 else nc.gpsimd
                    eng.dma_start(
                        out=xtr[:, e0:e0 + half, :],
                        in_=xT[e0 * 128:(e0 + half) * 128, s0:s0 + 512]
                        .rearrange("(c p) s -> p c s", p=128),
                    )
                return xtr

            # x tiles for sb 0/1 first on scalar/gpsimd queues; mask
            # prefetch ramps on the sync queue only during phase A.
            xtrs = {0: xtr_dma(0), 1: xtr_dma(1), 2: xtr_dma(2)}
            for G in range(2):
                mask_dma(G, nc.sync)

            with tc.tile_pool(name="psA", bufs=2, space="PSUM") as psA:
                for sb in range(NB):
                    for G in range(2 + sb * 2, 2 + sb * 2 + 2):
                        if G < MLOOK:
                            mask_dma(G, nc.sync)
                    if sb + 3 < NB:
                        xtrs[sb + 3] = xtr_dma(sb + 3)
                    xtr = xtrs.pop(sb)
                    # q/k/v head-major, 128-wide moving chunks (N=128
                    # chains stream ~0.44 ns/col vs 0.61 at N=512; the
                    # stationary w chunk is constant within a chain so its
                    # LDW hides).  Only the first chunk of each PSUM bank
                    # uses start=True: the 2KB zero-region covers the
                    # whole bank row and later chunks ride pending-zero.
                    q_ps = psA.tile([H, 512], F32, tag="qk", name=f"q_ps_{sb}",
                                    bufs=6)
                    k_ps = psA.tile([H, 512], F32, tag="qk", name=f"k_ps_{sb}",
                                    bufs=6)
                    v_ps = psA.tile([H, 512], F32, tag="qk", name=f"v_ps_{sb}",
                                    bufs=6)
                    for e in range(NE):
                        for ps, w_r in ((q_ps, wq_r), (k_ps, wk_r),
                                        (v_ps, wv_r)):
                            for c in range(4):
                                nc.tensor.matmul(
                                    ps[:, 128 * c:128 * c + 128],
                                    w_r[:, e, :],
                                    xtr[:, e, 128 * c:128 * c + 128],
                                    start=(e == 0 and c == 0),
                                    stop=(e == NE - 1),
                                )
                    nc.scalar.activation(QTb[sb], q_ps,
                                         mybir.ActivationFunctionType.Identity,
                                         bias=bqt_sb)
                    nc.scalar.activation(KTb[sb], k_ps,
                                         mybir.ActivationFunctionType.Identity,
                                         bias=bkt_sb)
                    # V' needs S-major layout: bias via ACT, then 4 PE
                    # transposes into one PSUM bank (sibling-safe pattern
                    # proven by the output fixup), ones column by memset.
                    vt = osb.tile([H, 512], F32, tag="vt")
                    nc.scalar.activation(vt, v_ps,
                                         mybir.ActivationFunctionType.Identity,
                                         bias=bvt_sb)
                    vt_ps = psA.tile([128, 4, H], F32, tag="vtp",
                                     name=f"vt_ps_{sb}", bufs=2)
                    for j in range(4):
                        nc.tensor.transpose(
                            vt_ps[:, j, :], vt[:, 128 * j:128 * (j + 1)],
                            idf[0:H, 0:H]
                        )
                    for j in range(4):
                        vp = VPk[sb * 4 + j]
                        nc.vector.tensor_copy(vp[:, 0:H], vt_ps[:, j, :])
                        nc.vector.memset(vp[:, H:H + 1], 1.0)

            # (A deliberate pre-phase-B idle to appease the activity
            # governor was tried and did NOT prevent clamping — the clamp
            # windows appear driven by package-level state outside this
            # kernel's control, so no time is wasted on idling.)

            # ---- phase B: flat pipeline over all half-groups ----
            # Each [128, 512] score half-tile is ONE PSUM bank; exp and
            # attn@V consume per half so banks recycle twice as fast as
            # the [128,1024] two-bank variant (which stalled scores ~315ns
            # per group on bank reuse).
            with (
                tc.tile_pool(name="psS", bufs=6, space="PSUM") as psS,
                tc.tile_pool(name="psO", bufs=1, space="PSUM") as psO,
            ):
                ot_ps = [None] * NB
                HQ = 2 * GQ  # half-groups; HG -> qb = HG//32, k = HG%32

                def scoresH(HG):
                    # NOTE: chunking these into 4x128-wide matmuls was
                    # tried (v9): the denser schedule trips the activity
                    # governor and the whole run clamps to ~half rate.
                    # The 512-wide form runs at the power-sustainable pace.
                    qb, k = HG // (2 * NG), HG % (2 * NG)
                    G, h2 = divmod(HG, 2)
                    sc = psS.tile([128, 512], F32, tag="sc", name=f"sc_{HG}")
                    mt = mtiles[G]
                    nc.tensor.matmul(
                        sc,
                        KTb[k // 4][:, 128 * (k % 4):128 * (k % 4 + 1)],
                        QTb[qb],
                        start=True, stop=False,
                    )
                    for j in range(4):
                        nc.tensor.matmul(
                            sc[:, 128 * j:128 * (j + 1)],
                            mt[:, j, h2, :], negI,
                            start=False, stop=(j == 3),
                        )
                    if h2 == 1:
                        mtiles.pop(G)
                    return sc

                def expH(HG, sc):
                    et = etp.tile([128, 512], BF16, tag="et")
                    nc.scalar.activation(
                        et, sc, mybir.ActivationFunctionType.Exp, scale=SCALE
                    )
                    return et

                def attnvH(HG, et):
                    qb, k = HG // (2 * NG), HG % (2 * NG)
                    if ot_ps[qb] is None:
                        ot_ps[qb] = psO.tile([H + 1, 512], F32, tag="ot",
                                             name=f"ot_{qb}")
                    nc.tensor.matmul(
                        ot_ps[qb], VPk[k], et,
                        start=(k == 0), stop=(k == NQ - 1),
                    )

                def fixup(qb):
                    q0 = qb * 512
                    oT = osb.tile([H + 1, 512], F32, tag="oT")
                    nc.vector.tensor_copy(oT, ot_ps[qb])
                    fx = psS.tile([128, 4, H + 1], F32, tag="fx", bufs=1)
                    for j in range(4):
                        nc.tensor.transpose(
                            fx[:, j, :], oT[:, 128 * j:128 * (j + 1)],
                            idf[0:H + 1, 0:H + 1]
                        )
                    ob = osb.tile([128, 4, H + 1], F32, tag="ob")
                    nc.vector.tensor_copy(ob, fx)
                    rc = osb.tile([128, 4], F32, tag="rc")
                    nc.vector.reciprocal(rc, ob[:, :, H])
                    of = osb.tile([128, 4, H], F32, tag="of")
                    for j in range(4):
                        nc.vector.tensor_scalar_mul(
                            of[:, j, :], ob[:, j, 0:H], rc[:, j:j + 1]
                        )
                    nc.gpsimd.dma_start(
                        out=out[q0:q0 + 512, :].rearrange("(j p) h -> p j h", p=128),
                        in_=of,
                    )

                KH = 2 * NG  # half-groups per qb
                ets = {}
                scn = {0: scoresH(0), 1: scoresH(1)}
                for HG in range(HQ):
                    if HG % 2 == 0 and HG // 2 + MLOOK < GQ:
                        mask_dma(HG // 2 + MLOOK,
                                 nc.sync if HG % 4 == 0 else nc.gpsimd)
                    if HG + 2 < HQ:
                        scn[HG + 2] = scoresH(HG + 2)
                    ets[HG] = expH(HG, scn.pop(HG))
                    if HG - HTRAIL >= 0:
                        attnvH(HG - HTRAIL, ets.pop(HG - HTRAIL))
                        if (HG - HTRAIL) % KH == KH - 1:
                            fixup((HG - HTRAIL) // KH)
                for HG in range(HQ - HTRAIL, HQ):
                    attnvH(HG, ets.pop(HG))
                    if HG % KH == KH - 1:
                        fixup(HG // KH)
    nc.compile()
    return nc


def make_in_maps(x, attention_mask, Wq, bq, Wk, bk, Wv, bv):
    nb = x.shape[0]
    NG, NB = S // 256, S // 512

    common = {
        "wq": np.ascontiguousarray(Wq.astype(BF16NP)),
        "wk": np.ascontiguousarray(Wk.astype(BF16NP)),
        "wv": np.ascontiguousarray(Wv.astype(BF16NP)),
        "bqt": np.ascontiguousarray(bq.reshape(H, 1)),
        "bkt": np.ascontiguousarray(bk.reshape(H, 1)),
        "bvt": np.ascontiguousarray(bv.reshape(H, 1)),
    }
    in_maps = []
    for b in range(nb):
        # mask -> fp8 {0,1} bytes (1.0 == 0x38 in e4m3), pre-tiled to
        # [(g, qb, p), j, h2, c]: mask[qb*512 + j*128 + p, (2g+h2)*128 + c]
        m8 = ((attention_mask[b] != 0).astype(np.uint8) * np.uint8(0x38))
        mt = m8.reshape(NB, 4, 128, NG, 2, 128).transpose(3, 0, 2, 1, 4, 5)
        mt = np.ascontiguousarray(mt).reshape(NG * NB * 128, 4, 2, 128)
        in_maps.append({
            "xT": np.ascontiguousarray(x[b].T.astype(BF16NP)),
            "maskt": mt.view(FP8NP),
            **common,
        })
    return in_maps


_PROGRAM = None


def kernel(x, attention_mask, Wq, bq, Wk, bk, Wv, bv):
    global _PROGRAM
    x = np.asarray(x, np.float32)
    attention_mask = np.asarray(attention_mask, np.int32)
    if _PROGRAM is None:
        _PROGRAM = build_program()
    in_maps = make_in_maps(
        x, attention_mask,
        np.asarray(Wq, np.float32), np.asarray(bq, np.float32),
        np.asarray(Wk, np.float32), np.asarray(bk, np.float32),
        np.asarray(Wv, np.float32), np.asarray(bv, np.float32),
    )
    res = run_bass_kernel_spmd(_PROGRAM, in_maps, core_ids=list(range(B)))
    return np.stack([res.results[b]["out"] for b in range(B)], axis=0)
